# revision 71
# baseline (speedup 1.0000x reference)
"""Bass/Tile TRN2 kernel for nn_MultiHeadAttention_549755814006.

Per-core work (data-parallel over batch, 8 cores, one batch element each):
  - L2-distance attention over 8 heads: softmax(-(|q-k|^2)/13) @ v
    Math: softmax_k(-(sq - 2 q.k + sk)/13) == softmax_k((2 q.k - sk)/13)
    (the per-query sq term cancels in softmax). We compute S^T = K @ Q^T on
    the PE (contraction over d=80 on partitions), exp on ACT with the
    per-key factor esk=exp(-sk/13) folded multiplicatively into the [V|1]
    weights, then out^T = [V*esk|esk]^T @ P^T which yields both the
    unnormalized head output (rows 0..79) and the softmax normalizer
    (row 80) in one accumulation.
  - fc projection accumulated over heads on PE directly from the transposed
    head outputs, with fc_b added via a rank-1 (ones x fc_b) matmul.
  - residual + LayerNorm epilogue in fp32.

Schedule (steady state is ACT-exp paced, PE co-saturated):
  - Loads are priority-ordered on the SWDGE FIFO (fc_w first for early WT
    prep, then q/k bf16, v interleaved between the k halves); the f32 q and
    epilogue constants ride the sync queue behind gate-writes so the list
    scheduler cannot front-run them onto the serial DMA-engine pool.
  - The attention stream is software-pipelined PIPE=5 slots deep: slot g
    emits S^T+exp for (h,t) and the attnV pair for (h,t-5), so neither the
    attnV-accumulator (po) slot wait nor a head boundary ever blocks the
    in-order PE queue ahead of the next S^T matmul.
  - PSUM: "spp" x2 ([128,1024]f32) holds the S^T rotation; "pop" holds the
    single-buffered attnV accumulator (po), two 1-bank transpose tiles
    (tpq/tpk) whose q/k transposes+evacs are dribbled 4-at-a-time between
    key-chunks, and the W^T prep (one head per attention head at t==5).
    8 banks exactly.
  - esk = exp(-sk/13) for all heads is batched in two key-tile halves
    before the exp stream starts (any ACT op inserted mid-stream costs its
    full duration); head 0's esk is split out first to unblock its vo prep.
  - Per-head softmax 1/s normalization rides a DRAM round-trip on the idle
    HWDGE; head 7 (tail-critical) too, feeding the fc's trailing matmuls.
  - fc runs in 6 small groups alternating psum slot families (spp pair vs
    po + tpq/tpk split pair) so consecutive groups share no psum and the
    PE streams the fc phase; the LN epilogue per l-tile uses ACT for
    evac+rowsum and Square+rowsum (residual rowsums are gated onto ACT's
    post-attention idle window), DVE dual-scalar ops for the normalize,
    and alternates the ln_w/ln_b ops across DVE/Pool with the final tile
    kept all-DVE for the shortest drain.

All matmuls in bf16 (fp32 matmul is 4x slower; fp8 gives no speedup on this
walrus build - DoubleRow fails codegen); final LN output is
residual-dominated (gamma_1 = 1e-4) so bf16 attention error is suppressed
by 1e4. The epilogue (residual add + LN) is fp32.
"""

import os
import sys
from contextlib import ExitStack

import numpy as np

for _p in (
    "/root/.axon_site",
    "/root/.axon_site/_ro/trn_rl_repo",
    "/root/.axon_site/_ro/pypackages",
    "/opt/trn_rl_repo",
):
    if os.path.isdir(_p) and _p not in sys.path:
        sys.path.append(_p)

import concourse.bass as bass
import concourse.mybir as mybir
import concourse.tile as tile
from concourse.bass_utils import run_bass_kernel_spmd

# ---------------------------------------------------------------------------
# This container's walrus build predates concourse's butterfly-barrier and
# EVENT_SEMAPHORE_RANGE_CLEAR emission - both fail codegen ("ISA wrong
# length" / setupSyncWait<CTRL_NO>). Patch bass/tile to emit the legacy
# PSEUDO_SYNC_BARRIER (expanded by NRT at load time) and skip the kernel-tail
# semaphore clear (sems are reinitialized per execution by the runtime;
# verified by repeat-execution tests).
# ---------------------------------------------------------------------------


def _patch_bass_for_old_walrus():
    if getattr(bass.Bass, "_old_walrus_patched", False):
        return

    def all_engine_barrier(self, *, sem_only=False):
        self._nrt_pseudo_barrier()

    def clear_and_free_semaphores(self, sems):
        return

    def _drain_and_barrier(self, tick_clock, wait_clock):
        self.nc.sync.drain()
        self.nc.all_engine_barrier()
        popped = self.nc._tile_sem_poison_stack.pop()
        assert popped is self._sem_poison
        self.nc.all_engine_barrier()

    bass.Bass.all_engine_barrier = all_engine_barrier
    bass.Bass.clear_and_free_semaphores = clear_and_free_semaphores
    tile.TileContext._drain_and_barrier = _drain_and_barrier
    bass.Bass._old_walrus_patched = True


_patch_bass_for_old_walrus()


def _split_multiwaits(nc):
    """This walrus encodes at most one semaphore wait per instruction.
    Move extra waits onto prefix NoOps on the same engine (sequentially
    blocking, so semantics are identical)."""
    k = 0
    for f in nc.m.functions:
        for blk in f.blocks:
            out = []
            for inst in blk.instructions:
                si = inst.sync_info
                waits = list(si.on_wait) if si is not None and si.on_wait else []
                if len(waits) > 1:
                    for w in waits[:-1]:
                        nop = mybir.InstNoOp(name=f"splitw-{k}")
                        k += 1
                        nop.engine = inst.engine
                        nop.sync_info = mybir.SyncInfo(on_wait=[w], on_update=[])
                        out.append(nop)
                    ups = list(si.on_update) if si.on_update else []
                    inst.sync_info = mybir.SyncInfo(on_wait=[waits[-1]], on_update=ups)
                out.append(inst)
            blk.instructions = out

B, L, H, DK, DM = 8, 1024, 8, 80, 640
NT = L // 128  # 8 row-tiles of 128 (both key-chunks and query/l-tiles)
NW = DM // 128  # 5 column blocks of fc_w
F32 = mybir.dt.float32
BF16 = mybir.dt.bfloat16
AF = mybir.ActivationFunctionType
ALU = mybir.AluOpType
LN_EPS = 1e-5


def _build_nc():
    nc = bass.Bass("TRN2")

    qd = nc.dram_tensor("q", [L, DM], F32, kind="ExternalInput")
    kd = nc.dram_tensor("k", [L, DM], F32, kind="ExternalInput")
    vd = nc.dram_tensor("v", [L, DM], F32, kind="ExternalInput")
    fwd = nc.dram_tensor("fc_w", [DM, DM], F32, kind="ExternalInput")
    fbd = nc.dram_tensor("fc_b", [DM], F32, kind="ExternalInput")
    gd = nc.dram_tensor("gamma_1", [DM], F32, kind="ExternalInput")
    lwd = nc.dram_tensor("ln_w", [DM], F32, kind="ExternalInput")
    lbd = nc.dram_tensor("ln_b", [DM], F32, kind="ExternalInput")
    od = nc.dram_tensor("out", [L, DM], F32, kind="ExternalOutput")

    with ExitStack() as ctx:
        tc = ctx.enter_context(
            tile.TileContext(nc, trace_sim=os.environ.get("KERNEL_TRACE_SIM") == "1")
        )

        singles = ctx.enter_context(tc.tile_pool(name="singles", bufs=1))
        loads = ctx.enter_context(tc.tile_pool(name="loads", bufs=8))
        wt_pool = ctx.enter_context(tc.tile_pool(name="wt", bufs=8))
        qt_pool = ctx.enter_context(tc.tile_pool(name="qt", bufs=2))
        sk_pool = ctx.enter_context(tc.tile_pool(name="sk", bufs=2))
        vo_pool = ctx.enter_context(tc.tile_pool(name="vo", bufs=16))
        pt_pool = ctx.enter_context(tc.tile_pool(name="pt", bufs=10))
        ot_pool = ctx.enter_context(tc.tile_pool(name="ot", bufs=8))
        r_pool = ctx.enter_context(tc.tile_pool(name="r", bufs=2))
        e_pool = ctx.enter_context(tc.tile_pool(name="epi", bufs=2))
        s_pool = ctx.enter_context(tc.tile_pool(name="stats", bufs=8))
        # PSUM: "sp" = S^T tiles [128,1024]f32 (2 banks) x2 bufs = 4 banks,
        # also q/k transposes + 2 fc accumulators; "pop" = attnV accumulators
        # [81,1024]f32 (2 banks) x2 bufs = 4 banks, also pw + 2 fc
        # accumulators. Total exactly 8 banks.
        spp = ctx.enter_context(tc.tile_pool(name="spp", bufs=2, space="PSUM"))
        pop = ctx.enter_context(tc.tile_pool(name="pop", bufs=2, space="PSUM"))
        dram = ctx.enter_context(tc.tile_pool(name="dram", bufs=2, space="DRAM"))

        # ---------------- constants ----------------
        ident_dram = nc.inline_tensor(
            np.eye(128, dtype=np.float32).astype(__import__("ml_dtypes").bfloat16),
            name="ident128",
        )
        ident = singles.tile([128, 128], BF16, tag="ident")
        nc.sync.dma_start(out=ident, in_=ident_dram[:, :])

        ones1 = singles.tile([1, 128], BF16, tag="ones1")
        nc.vector.memset(ones1, 1.0)
        ones80 = singles.tile([1, DK], BF16, tag="ones80")
        nc.vector.memset(ones80, 1.0)
        epsb = singles.tile([128, 1], F32, tag="epsb")
        nc.vector.memset(epsb, float(LN_EPS))

        # ---------------- loads (priority order) ----------------
        # fc_w first (early W^T prep), then q/k bf16 (first head), v, then
        # f32 q (epilogue-only). SWDGE casts f32->bf16 in flight. The DMA
        # engine pool is a serial resource; program order here is transfer
        # order.
        NH = NT // 2
        qb_all = loads.tile([128, NT, DM], BF16, tag="qb", bufs=1)
        qdv = qd.rearrange("(t p) d -> p t d", p=128)
        nc.gpsimd.dma_start(out=qb_all, in_=qdv[:, :, :])
        kb_all = loads.tile([128, NT, DM], BF16, tag="kb", bufs=1)
        kdv = kd.rearrange("(t p) d -> p t d", p=128)
        nc.gpsimd.dma_start(out=kb_all[:, 0:NH, :], in_=kdv[:, 0:NH, :])
        # v's first half lands between the k halves: the first attnV pairs
        # need it ~3us before the second k half is needed by exp(0,4)
        vb_all = loads.tile([128, NT, DM], BF16, tag="vb", bufs=1)
        vdv = vd.rearrange("(t p) d -> p t d", p=128)
        nc.gpsimd.dma_start(out=vb_all[:, 0:NH, :], in_=vdv[:, 0:NH, :])
        nc.gpsimd.dma_start(out=kb_all[:, NH:NT, :], in_=kdv[:, NH:NT, :])
        nc.gpsimd.dma_start(out=vb_all[:, NH:NT, :], in_=vdv[:, NH:NT, :])
        # fc_w/gamma after the attention operands: W^T prep is dribbled into
        # the head loop and first needs them only at ~24us
        fwb_all = loads.tile([128, NW, DM], BF16, tag="fwb", bufs=1)
        nc.gpsimd.dma_start(out=fwb_all, in_=fwd.rearrange("(j p) d -> p j d", p=128))
        gammaB = singles.tile([128, DM], F32, tag="gammaB")
        nc.scalar.dma_start(out=gammaB, in_=gd.reshape([1, DM]).broadcast_to([128, DM]))
        # qf (f32 residual) + epilogue constants load on the sync queue,
        # emitted mid-attention behind the head-0/2 normalize round-trips so
        # they can neither preempt the bulk loads on the serial DMA-engine
        # pool nor stuff the SWDGE ring (which would block Pool's vo ops).
        qf_all = loads.tile([128, NT, DM], F32, tag="qf", bufs=1)
        fcb_b = singles.tile([1, DM], F32, tag="fcbb")
        lnwB = singles.tile([128, DM], F32, tag="lnwB")
        lnbB = singles.tile([128, DM], F32, tag="lnbB")
        qb = [qb_all[:, t, :] for t in range(NT)]
        kb = [kb_all[:, t, :] for t in range(NT)]
        vb = [vb_all[:, t, :] for t in range(NT)]
        qf = [qf_all[:, t, :] for t in range(NT)]
        fwb = [fwb_all[:, j, :] for j in range(NW)]

        # ---------------- W^T prep (before attention) ----------------
        # WT[h][d, o] = fc_w[o, h*80+d] with gamma_1 folded in, bf16.
        # pw tiles lead the S^T psum rotation; the gamma fold is an all-bf16
        # DVE mul (2x mode, ~390ns) so the 8-slot ping-pong chain completes
        # before the first S^T matmul needs the pool (~10us).
        gammaBb = singles.tile([128, DM], BF16, tag="gammaBb")
        with nc.allow_low_precision(reason="gamma fold in bf16; scales the 1e-4-suppressed path"):
            nc.vector.tensor_copy(gammaBb, gammaB)
        WT = {}

        def build_wt(wh, tag):
            # one head of W^T per attention head, dribbled into the briefly
            # idle transpose psum slot at t==5 (WT is needed only by the fc
            # tail); gamma_1 folded in bf16.
            hs = slice(wh * DK, (wh + 1) * DK)
            pw = pop.tile([DK, DM], BF16, tag=tag, name=f"pw{wh}", bufs=1)
            for j in range(NW):
                nc.tensor.transpose(pw[:, j * 128 : (j + 1) * 128], fwb[j][:, hs], ident)
            w = wt_pool.tile([DK, DM], BF16, tag="wt", name=f"wt{wh}")
            with nc.allow_low_precision(reason="bf16 W^T gamma fold"):
                nc.vector.tensor_mul(w, pw, gammaBb[0:DK, :])
            WT[wh] = w

        # ---------------- attention, head by head ----------------
        # per-key factor esk = exp(-sk/13) for ALL heads, batched in two
        # key-tile halves before the exp stream starts (any ACT op inserted
        # mid-stream costs its full duration since ACT is the pacer). k^2 in
        # bf16 on DVE (2x mode); the esk error is gamma-suppressed.
        scr_all = sk_pool.tile([128, NT, DM], BF16, tag="skscr", bufs=1)
        skb_all = sk_pool.tile([128, H, NT], F32, tag="skb", bufs=1)
        eskb_all = sk_pool.tile([128, H, NT], F32, tag="eskb", bufs=1)

        def esk_batch(half):
            ts_ = slice(half * NH, (half + 1) * NH)
            with nc.allow_low_precision(reason="k^2 in bf16; esk error is gamma-suppressed"):
                nc.vector.tensor_mul(scr_all[:, ts_, :], kb_all[:, ts_, :], kb_all[:, ts_, :])
            # head 0's esk first - it gates the first attnV pairs
            for h in range(H):
                hs = slice(h * DK, (h + 1) * DK)
                nc.vector.tensor_reduce(
                    skb_all[:, h, ts_], scr_all[:, ts_, hs],
                    axis=mybir.AxisListType.X, op=ALU.add,
                )
                if h == 0:
                    nc.scalar.activation(eskb_all[:, 0:1, ts_], skb_all[:, 0:1, ts_],
                                         AF.Exp, bias=0.0, scale=-1.0 / 13.0)
            nc.scalar.activation(eskb_all[:, 1:H, ts_], skb_all[:, 1:H, ts_],
                                 AF.Exp, bias=0.0, scale=-1.0 / 13.0)

        # The 16 q/k transposes per head land in four dedicated 1-bank psum
        # tiles (half-planes), each with its own evac. This decouples the
        # first S^T matmuls from the second half of the k load, keeps the
        # S^T psum rotation pure, and never blocks on the po accumulator.
        # part: 0 = q tiles 0-3, 1 = q tiles 4-7, 2 = k tiles 0-3, 3 = k 4-7.
        tp_tiles = {}

        def prep_part(h, part):
            hs = slice(h * DK, (h + 1) * DK)
            u, half = divmod(part, 2)
            if (h, u) not in tp_tiles:
                tp_tiles[(h, u)] = pop.tile(
                    [DK, L], BF16, tag=("tpq" if u == 0 else "tpk"), name=f"tp{u}_{h}", bufs=1
                )
            pp = tp_tiles[(h, u)]
            for i in range(4):
                t = half * 4 + i
                src = qb[t] if u == 0 else kb[t]
                nc.tensor.transpose(pp[:, t * 128 : (t + 1) * 128], src[:, hs], ident)
            hc = slice(half * 512, (half + 1) * 512)
            sb = qt_pool.tile([DK, 512], BF16, tag=f"qk{part}", name=f"qk{part}_{h}")
            nc.vector.tensor_copy(sb, pp[:, hc])
            return sb

        def prep_vo(h, tiles):
            """[V*esk|esk] tiles for head h (Pool)."""
            hs = slice(h * DK, (h + 1) * DK)
            vos = []
            for t in tiles:
                e1 = eskb_all[:, h, t : t + 1]
                vo = vo_pool.tile([128, DK + 1], BF16, tag="vo")
                nc.gpsimd.tensor_mul(vo[:, 0:DK], vb[t][:, hs], e1.broadcast_to([128, DK]))
                nc.gpsimd.tensor_copy(vo[:, DK : DK + 1], e1)
                vos.append(vo)
            return vos

        # The attention stream is software-pipelined one slot deep: at global
        # slot g we emit S^T+exp for (h,t) and the attnV pair for (h,t-1).
        # At a head boundary the next head's first S^T therefore reaches the
        # PE queue BEFORE the previous head's last two attnV pairs, so exp
        # never waits out the attnV+S^T serial chain (~1.3us/head saved).
        oTs = []
        late_gates = []
        preps = {0: [prep_part(0, p) for p in range(4)]}
        esk_batch(0)
        esk_batch(1)
        vos = {0: prep_vo(0, range(NH)) + prep_vo(0, range(NH, NT))}
        pos = {}
        pts = {}

        def emit_attnv(h, t):
            for qc in (0, 512):
                nc.tensor.matmul(
                    pos[h][:, qc : qc + 512],
                    vos[h][t],
                    pts[(h, t)][:, qc : qc + 512],
                    start=(t == 0),
                    stop=(t == NT - 1),
                )
            del pts[(h, t)]

        def finish_head(h):
            """Evacuate + normalize head h right after its last attnV."""
            po = pos.pop(h)
            oT = ot_pool.tile([DK, L], BF16, tag="oT", name=f"oT{h}")
            if h < H - 1:
                # r = 1/s broadcast over the 80 d-partitions via a DRAM
                # round-trip on the near-idle HWDGE ring; fully hidden under
                # subsequent heads. Evac to SBUF first so the slot frees.
                oTu = r_pool.tile([DK + 1, L], F32, tag="oTu")
                nc.vector.tensor_copy(oTu, po)
                sscr = dram.tile([1, L], F32, tag="sscr")
                nc.sync.dma_start(out=sscr, in_=oTu[DK : DK + 1, :])
                scols = r_pool.tile([128, NT], F32, tag="scols")
                nc.sync.dma_start(out=scols, in_=sscr.rearrange("a (t p) -> (a p) t", p=128))
                rcols = r_pool.tile([128, NT], F32, tag="rcols")
                nc.vector.reciprocal(rcols, scols)
                if h == H - 2:
                    late_gates.append(scols)
                rscr = dram.tile([1, L], F32, tag="rscr")
                nc.sync.dma_start(out=rscr.rearrange("a (t p) -> (a p) t", p=128), in_=rcols)
                rb = r_pool.tile([DK, L], F32, tag="rb")
                nc.sync.dma_start(out=rb, in_=rscr[0:1, :].broadcast_to([DK, L]))
                eng = nc.gpsimd if h == H - 2 else nc.vector
                eng.tensor_mul(oT, oTu[0:DK, :], rb)  # head 6 on Pool: keeps
                # DVE free for head 7's tail-critical chain
            else:
                # head 7 is tail-critical: a 2-hop DMA brings the s-row to
                # partition 0, then a partition-0 reciprocal, a rank-1 PE
                # broadcast of 1/s into the freed S^T psum slot, and the mul
                oTu = r_pool.tile([DK + 1, L], F32, tag="oTu")
                nc.vector.tensor_copy(oTu, po)
                sscr = dram.tile([1, L], F32, tag="sscr")
                nc.sync.dma_start(out=sscr, in_=oTu[DK : DK + 1, :])
                srow0 = r_pool.tile([1, L], F32, tag="srow0")
                nc.sync.dma_start(out=srow0, in_=sscr[0:1, :])
                rrow = r_pool.tile([1, L], BF16, tag="rrow")
                with nc.allow_low_precision(reason="1/s broadcast in bf16; attention path is gamma-suppressed by 1e-4"):
                    nc.vector.reciprocal(rrow, srow0)
                rp = spp.tile([DK, L], F32, tag="big", name="rbc")
                nc.tensor.matmul(rp[:, 0:512], ones80, rrow[:, 0:512], start=True, stop=True)
                nc.tensor.matmul(rp[:, 512:1024], ones80, rrow[:, 512:1024], start=True, stop=True)
                nc.vector.tensor_mul(oT, oTu[0:DK, :], rp)
            oTs.append(oT)

        for g in range(H * NT):
            h, t = divmod(g, NT)
            if t == 0:
                pos[h] = pop.tile([DK + 1, L], F32, tag="po", name=f"po{h}", bufs=1)
                preps.setdefault(h, [None] * 4)
            qTh = preps[h][0:2]  # [DK, 512] halves of Q^T
            kTh = preps[h][2:4]
            ps = spp.tile([128, L], F32, tag="big")
            kTt = kTh[t // 4][:, (t % 4) * 128 : (t % 4 + 1) * 128]
            nc.tensor.matmul(ps[:, 0:512], kTt, qTh[0], start=True, stop=True)
            nc.tensor.matmul(ps[:, 512:1024], kTt, qTh[1], start=True, stop=True)
            pt = pt_pool.tile([128, L], BF16, tag="pt")
            nc.scalar.activation(out=pt, in_=ps, func=AF.Exp, bias=0.0, scale=2.0 / 13.0)
            pts[(h, t)] = pt
            if g >= PIPE:
                ph, pt_ = divmod(g - PIPE, NT)
                emit_attnv(ph, pt_)
                if pt_ == NT - 1:
                    finish_head(ph)
            if h + 1 < H:
                if 1 <= t <= 4:
                    preps.setdefault(h + 1, [None] * 4)[t - 1] = prep_part(h + 1, t - 1)
                if t == 5:
                    vos[h + 1] = prep_vo(h + 1, range(NT))
                    build_wt(h, "tpq")
                    if h == 0:
                        build_wt(7, "tpk")
            if g == NT + 1:
                # gate-write: a 1-element copy whose WAW hazard with the DMA
                # forcibly delays its issue until attention is underway
                # (the list scheduler otherwise front-runs ready DMAs and
                # the 7us f32-q transfer would preempt the bulk loads)
                nc.vector.tensor_copy(qf_all[0:1, 0, 0:1], eskb_all[0:1, 0, 0:1])
                nc.sync.dma_start(out=qf_all, in_=qd.rearrange("(t p) d -> p t d", p=128))
            if g == 3 * NT + 1:
                nc.vector.tensor_copy(fcb_b[0:1, 0:1], qf_all[0:1, 0, 0:1])
                nc.sync.dma_start(out=fcb_b, in_=fbd.reshape([1, DM])[:, :])
                nc.vector.tensor_copy(lnwB[0:1, 0:1], fcb_b[0:1, 0:1])
                nc.sync.dma_start(out=lnwB, in_=lwd.reshape([1, DM]).broadcast_to([128, DM]))
                nc.vector.tensor_copy(lnbB[0:1, 0:1], lnwB[0:1, 0:1])
                nc.sync.dma_start(out=lnbB, in_=lbd.reshape([1, DM]).broadcast_to([128, DM]))
        for t_ in range(NT - PIPE, NT):
            emit_attnv(H - 1, t_)
        finish_head(H - 1)

        # residual row-sums for the LN mean, on ACT in its idle window
        # between the last exp and the first fc evac. The zero-valued bias
        # AP is written by an op gated on head 6's normalize, which stops
        # the scheduler hoisting these into the exp stream (ACT is the
        # attention pacer).
        gate0 = s_pool.tile([128, 1], F32, tag="gate0")
        nc.vector.tensor_scalar_mul(gate0, late_gates[0][:, 0:1], 0.0)
        sumqs = []
        for lt in range(NT):
            sq_ = s_pool.tile([128, 1], F32, tag="sumq", name=f"sumq{lt}", bufs=8)
            qscr = e_pool.tile([128, DM], F32, tag="qscr", bufs=1)
            nc.scalar.activation(qscr, qf[lt], AF.Identity, bias=gate0, scale=1.0, accum_out=sq_)
            sumqs.append(sq_)

        def _ep_evac(lts, ypss):
            # psum evacs: the next group-pair's accumulator slots wait on
            # these, so they're emitted separately from the LN chains
            t1s, sumts = {}, {}
            for lt in lts:
                t1s[lt] = e_pool.tile([128, DM], F32, tag="t1", name=f"t1_{lt}", bufs=3)
                y = ypss[lt]
                # PSUM evac (gamma already folded into WT) + free row-sum
                if len(y) == 1:
                    sumts[lt] = [s_pool.tile([128, 1], F32, tag="sumt", name=f"sumt_{lt}")]
                    nc.scalar.activation(t1s[lt], y[0], AF.Identity, bias=0.0, scale=1.0,
                                         accum_out=sumts[lt][0])
                else:
                    sumts[lt] = [
                        s_pool.tile([128, 1], F32, tag="sumt", name=f"sumtA_{lt}"),
                        s_pool.tile([128, 1], F32, tag="sumtB", name=f"sumtB_{lt}"),
                    ]
                    nc.scalar.activation(t1s[lt][:, 0:512], y[0], AF.Identity, bias=0.0,
                                         scale=1.0, accum_out=sumts[lt][0])
                    nc.scalar.activation(t1s[lt][:, 512:DM], y[1], AF.Identity, bias=0.0,
                                         scale=1.0, accum_out=sumts[lt][1])
            return t1s, sumts

        def _ep_chain(lts, t1s, sumts):
            for lt in lts:
                ls = slice(lt * 128, (lt + 1) * 128)
                t1 = t1s[lt]
                x = e_pool.tile([128, DM], F32, tag="x", bufs=3)
                (nc.gpsimd if lt == 6 else nc.vector).tensor_add(x, t1, qf[lt])  # + residual
                sumx = s_pool.tile([128, 1], F32, tag="sumx")
                if len(sumts[lt]) > 1:
                    sumth = s_pool.tile([128, 1], F32, tag="sumth")
                    nc.vector.tensor_add(sumth, sumts[lt][0], sumts[lt][1])
                    nc.vector.tensor_add(sumx, sumth, sumqs[lt])
                else:
                    nc.vector.tensor_add(sumx, sumts[lt][0], sumqs[lt])

                sq = e_pool.tile([128, DM], F32, tag="sq", bufs=1)
                sumsq = s_pool.tile([128, 1], F32, tag="sumsq")
                nc.scalar.activation(sq, x, AF.Square, bias=0.0, scale=1.0, accum_out=sumsq)
                mean = s_pool.tile([128, 1], F32, tag="mean")
                nc.vector.tensor_scalar_mul(mean, sumx, 1.0 / DM)
                msq = s_pool.tile([128, 1], F32, tag="msq")
                nc.vector.tensor_mul(msq, mean, mean)
                # var = sumsq/DM - mean^2 in one dual-scalar op
                var = s_pool.tile([128, 1], F32, tag="var")
                nc.vector.tensor_scalar(var, sumsq, 1.0 / DM, msq, op0=ALU.mult, op1=ALU.subtract)
                std = s_pool.tile([128, 1], F32, tag="std")
                nc.scalar.activation(std, var, AF.Sqrt, bias=epsb, scale=1.0)
                rstd = s_pool.tile([128, 1], F32, tag="rstd")
                nc.vector.reciprocal(rstd, std)
                msr = s_pool.tile([128, 1], F32, tag="msr")
                nc.vector.tensor_mul(msr, mean, rstd)

                # xn = (x - mean) * rstd = x*rstd - mean*rstd, one dual-scalar op
                xn = e_pool.tile([128, DM], F32, tag="xn", bufs=2)
                nc.vector.tensor_scalar(xn, x, rstd, msr, op0=ALU.mult, op1=ALU.subtract)
                y1 = e_pool.tile([128, DM], F32, tag="y1", bufs=2)
                y2 = e_pool.tile([128, DM], F32, tag="y2", bufs=3)
                if lt == 7:
                    # very last tile: keep the end-chain entirely on DVE
                    # (727ns ops vs 1365 on Pool)
                    nc.vector.tensor_mul(y1, xn, lnwB)
                    nc.vector.tensor_add(y2, y1, lnbB)
                elif lt == 6:
                    # second-to-last: both on Pool so DVE is clear for lt 7
                    nc.gpsimd.tensor_mul(y1, xn, lnwB)
                    nc.gpsimd.tensor_add(y2, y1, lnbB)
                elif lt % 2 == 0:
                    nc.vector.tensor_mul(y1, xn, lnwB)
                    nc.gpsimd.tensor_add(y2, y1, lnbB)
                else:
                    nc.gpsimd.tensor_mul(y1, xn, lnwB)
                    nc.vector.tensor_add(y2, y1, lnbB)
                nc.sync.dma_start(out=od[ls, :], in_=y2)

        fcb_g = singles.tile([1, DM], BF16, tag="fcbg")
        nc.vector.tensor_mul(fcb_g, fcb_b, gammaB[0:1, :])

        # ---------------- fc + residual + LayerNorm ----------------
        # Four groups of 2 l-tiles in two slot families (even groups: the
        # two spp slots; odd groups: the po slot + the tp-split pair), so
        # consecutive groups share no psum and the PE streams matmuls while
        # epilogues drain behind. Heads 0..6 first; head 7's (latest,
        # round-trip-normalized) output is needed only by the trailing
        # matmul of each group.
        def alloc_group(gi, lts):
            ypss = {}
            for i, lt in enumerate(lts):
                if gi % 2 == 0:
                    ypss[lt] = [spp.tile([128, DM], F32, tag="big", name=f"yps{lt}")]
                elif i == 0:
                    ypss[lt] = [pop.tile([128, DM], F32, tag="po", name=f"yps{lt}", bufs=1)]
                else:
                    lo = pop.tile([128, 512], F32, tag="tpq", name=f"ylo{lt}", bufs=1)
                    hi = pop.tile([128, DM - 512], F32, tag="tpk", name=f"yhi{lt}", bufs=1)
                    ypss[lt] = [lo, hi]
            return ypss

        def acc_slices(ypss, lt):
            y = ypss[lt]
            if len(y) == 1:
                return [(y[0][:, 0:512], slice(0, 512)), (y[0][:, 512:DM], slice(512, DM))]
            return [(y[0][:, :], slice(0, 512)), (y[1][:, :], slice(512, DM))]

        def mm_group(ypss, lts, heads, bias=False):
            for h in heads:
                for lt in lts:
                    ls = slice(lt * 128, (lt + 1) * 128)
                    for acc, cs in acc_slices(ypss, lt):
                        nc.tensor.matmul(acc, oTs[h][:, ls], WT[h][:, cs],
                                         start=(h == 0), stop=False)
            if bias:
                for lt in lts:
                    for acc, cs in acc_slices(ypss, lt):
                        nc.tensor.matmul(acc, ones1, fcb_g[:, cs], start=False, stop=True)

        h06 = list(range(H - 1))
        ys = {}
        for gi, lts in enumerate(GROUPS):
            ys[gi] = alloc_group(gi, lts)
            mm_group(ys[gi], lts, h06)
            mm_group(ys[gi], lts, [H - 1], bias=True)
            ts_ = _ep_evac(lts, ys[gi])
            _ep_chain(lts, *ts_)

    _split_multiwaits(nc)
    return nc


_cache = {}


def _get_nc():
    if "nc" not in _cache:
        _cache["nc"] = _build_nc()
    return _cache["nc"]


def _in_maps(q, k, v, fc_w, fc_b, gamma_1, ln_w, ln_b):
    q = np.ascontiguousarray(q, dtype=np.float32)
    k = np.ascontiguousarray(k, dtype=np.float32)
    v = np.ascontiguousarray(v, dtype=np.float32)
    fc_w = np.ascontiguousarray(fc_w, dtype=np.float32)
    fc_b = np.ascontiguousarray(fc_b, dtype=np.float32)
    gamma_1 = np.ascontiguousarray(gamma_1, dtype=np.float32)
    ln_w = np.ascontiguousarray(ln_w, dtype=np.float32)
    ln_b = np.ascontiguousarray(ln_b, dtype=np.float32)
    return [
        {
            "q": np.ascontiguousarray(q[b]),
            "k": np.ascontiguousarray(k[b]),
            "v": np.ascontiguousarray(v[b]),
            "fc_w": fc_w,
            "fc_b": fc_b,
            "gamma_1": gamma_1,
            "ln_w": ln_w,
            "ln_b": ln_b,
        }
        for b in range(B)
    ]


def kernel(q, k, v, fc_w, fc_b, gamma_1, ln_w, ln_b):
    nc = _get_nc()
    res = run_bass_kernel_spmd(
        nc, _in_maps(q, k, v, fc_w, fc_b, gamma_1, ln_w, ln_b),
        core_ids=list(range(B)),
    )
    return np.stack([r["out"] for r in res.results], axis=0)


def _build_null_nc():
    """Same I/O signature, DMA passthrough only — for dispatch-overhead calibration."""
    nc = bass.Bass("TRN2")
    qd = nc.dram_tensor("q", [L, DM], F32, kind="ExternalInput")
    for nm, shp in [("k", [L, DM]), ("v", [L, DM]), ("fc_w", [DM, DM]),
                    ("fc_b", [DM]), ("gamma_1", [DM]), ("ln_w", [DM]), ("ln_b", [DM])]:
        nc.dram_tensor(nm, shp, F32, kind="ExternalInput")
    od = nc.dram_tensor("out", [L, DM], F32, kind="ExternalOutput")
    with ExitStack() as ctx:
        tc = ctx.enter_context(tile.TileContext(nc))
        pool = ctx.enter_context(tc.tile_pool(name="p", bufs=4))
        for t in range(NT):
            rs = slice(t * 128, (t + 1) * 128)
            tt = pool.tile([128, DM], F32, tag="t")
            nc.sync.dma_start(out=tt, in_=qd[rs, :])
            nc.sync.dma_start(out=od[rs, :], in_=tt)
    _split_multiwaits(nc)
    return nc


def _pjrt_chain_callable(nc, chain):
    """Build a jitted fn that executes the NEFF `chain` times back-to-back
    in one dispatch, feeding each output back as the next q. Timing two
    chain lengths isolates per-execution device time from dispatch cost."""
    import jax
    from jax.sharding import Mesh, PartitionSpec, NamedSharding
    from jax.experimental.shard_map import shard_map
    from concourse import bass2jax, mybir as mb

    bass2jax.install_neuronx_cc_hook()
    in_names, out_names, out_avals, zero_outs = [], [], [], []
    for alloc in nc.m.functions[0].allocations:
        if not isinstance(alloc, mb.MemoryLocationSet):
            continue
        name = alloc.memorylocations[0].name
        if alloc.kind == "ExternalInput":
            in_names.append(name)
        elif alloc.kind == "ExternalOutput":
            out_names.append(name)
            shape = tuple(alloc.tensor_shape)
            dtype = mb.dt.np(alloc.dtype)
            out_avals.append(jax.core.ShapedArray(shape, dtype))
            zero_outs.append(np.zeros(shape, dtype))
    n_params = len(in_names)
    all_names = in_names + out_names
    qi = in_names.index("q")

    def _body(*args):
        outs = bass2jax._bass_exec_p.bind(
            *list(args),
            out_avals=tuple(out_avals),
            in_names=tuple(all_names),
            out_names=tuple(out_names),
            lowering_input_output_aliases=(),
            sim_require_finite=True,
            sim_require_nnan=True,
            nc=nc,
        )
        return tuple(outs)

    devices = jax.devices()[:B]
    mesh = Mesh(np.asarray(devices), ("core",))
    nshard = NamedSharding(mesh, PartitionSpec("core"))
    in_specs = (PartitionSpec("core"),) * (n_params + len(out_names))
    out_specs = (PartitionSpec("core"),) * len(out_names)
    fn = jax.jit(shard_map(_body, mesh=mesh, in_specs=in_specs,
                           out_specs=out_specs, check_rep=False), keep_unused=True)
    return fn, in_names, zero_outs, nshard


def bench(q, k, v, fc_w, fc_b, gamma_1, ln_w, ln_b, reps=15, chain=8):
    """Returns (output, per_exec_ns, t1_ns): per-NEFF-execution device time
    from the (chain vs 1) wall difference, plus single-dispatch wall."""
    import jax, time

    in_maps = _in_maps(q, k, v, fc_w, fc_b, gamma_1, ln_w, ln_b)
    nc = _get_nc()

    fn, in_names, zero_outs, nshard = _pjrt_chain_callable(nc, 1)
    qi = in_names.index("q")
    concat_in = []
    for nm in in_names:
        if nm == "partition_id":
            concat_in.append(np.arange(B, dtype=np.uint32).reshape(B, 1))
        else:
            concat_in.append(
                np.concatenate([np.asarray(in_maps[c][nm]) for c in range(B)], axis=0)
            )
    concat_zero = [np.zeros((B * z.shape[0], *z.shape[1:]), z.dtype) for z in zero_outs]
    dev_in = [jax.device_put(a, nshard) for a in concat_in + concat_zero]
    out1 = fn(*dev_in)
    jax.block_until_ready(out1)

    def timed(chain_n):
        times = []
        args = list(dev_in)
        for _ in range(reps):
            t0 = time.perf_counter()
            o = fn(*args)
            for _ in range(chain_n - 1):
                a2 = list(args)
                a2[qi] = o[0]
                o = fn(*a2)
            jax.block_until_ready(o)
            times.append(time.perf_counter() - t0)
        return min(times) * 1e9

    t1 = timed(1)
    tk = timed(chain)
    slope = (tk - t1) / (chain - 1)

    if "null" not in _cache:
        _cache["null"] = _build_null_nc()
    fn_n, in_names_n, zero_n, nshard_n = _pjrt_chain_callable(_cache["null"], 1)
    qi_n = in_names_n.index("q")
    ci = []
    for nm in in_names_n:
        if nm == "partition_id":
            ci.append(np.arange(B, dtype=np.uint32).reshape(B, 1))
        else:
            ci.append(np.concatenate([np.asarray(in_maps[c][nm]) for c in range(B)], axis=0))
    cz = [np.zeros((B * z.shape[0], *z.shape[1:]), z.dtype) for z in zero_n]
    dev_in_n = [jax.device_put(a, nshard_n) for a in ci + cz]
    jax.block_until_ready(fn_n(*dev_in_n))

    def timed_null(chain_n):
        times = []
        for _ in range(reps):
            t0 = time.perf_counter()
            o = fn_n(*dev_in_n)
            for _ in range(chain_n - 1):
                a2 = list(dev_in_n)
                a2[qi_n] = o[0]
                o = fn_n(*a2)
            jax.block_until_ready(o)
            times.append(time.perf_counter() - t0)
        return min(times) * 1e9

    tn1 = timed_null(1)
    tnk = timed_null(chain)
    slope_null = (tnk - tn1) / (chain - 1)

    per_exec = slope - slope_null
    res = np.asarray(out1[0]).reshape(B, L, DM)
    return res, per_exec, slope_null
